# revision 6
# baseline (speedup 1.0000x reference)
"""Trainium2 Bass kernel for hierarchical LSTM decoder (MusicVAE-style).

Strategy: tensor-parallel over the 4H gate dimension across 8 cores
(512 gate rows = 128 hidden units per core per layer). All recurrent
weights stay resident in SBUF (they do not fit on fewer cores in fp32).
Per recurrent step: each core computes gates for its hidden chunk with
float32r matmuls (activations stationary, 2-way column-tiled weight
streams), applies the LSTM cell in transposed (hidden-major) layout,
then all-gathers the 128x32 h-chunk so every core has the full hidden
state for the next contraction. The conductor LSTM (depends only on
`latent`) is computed up front the same way; its `emb` outputs and the
per-subsequence emb/bias gate contributions are precomputed into base
tiles. The feedback projection `prev` is computed redundantly on every
core from the gathered h1 (cheaper than a second reduce).
"""
import sys
import numpy as np

sys.path.insert(0, "/opt/trn_rl_repo")

import concourse.bass as bass  # noqa: E402
import concourse.bacc as bacc  # noqa: E402
import concourse.mybir as mybir  # noqa: E402
from concourse import tile  # noqa: E402
from concourse import bass_utils  # noqa: E402

F32R = mybir.dt.float32r
F32 = mybir.dt.float32
AF = mybir.ActivationFunctionType

B, LAT, CH, CO, INP, H, SEQ, NSUB, NL = 32, 512, 1024, 512, 389, 1024, 128, 8, 2
STEPS = SEQ // NSUB
INPP = 392  # INP padded to 4*98
R = 8  # cores / TP degree
KP = 98  # prev-part K chunk size


def _shard_gates(W, r):
    """W (4096, K) -> dev (K, 512) for core r.

    Device col t*128+n <- gate row t*1024 + r*128 + n, so transpose block t
    is gate t's (hidden, batch) tile.
    """
    K = W.shape[1]
    out = np.empty((K, 512), np.float32)
    for t in range(4):
        blk = W[t * 1024 + r * 128: t * 1024 + (r + 1) * 128, :]
        out[:, t * 128:(t + 1) * 128] = blk.T
    return out


def _shard_bias(b, r):
    out = np.empty((1, 512), np.float32)
    for t in range(4):
        out[0, t * 128:(t + 1) * 128] = b[t * 1024 + r * 128: t * 1024 + r * 128 + 128]
    return out


def _chunk_k(dev, nk, kp, perm=None):
    """dev (K, ...) -> (nk, kp, ...); optional chunk permutation."""
    K = dev.shape[0]
    assert K == nk * kp
    out = dev.reshape((nk, kp) + dev.shape[1:])
    if perm is not None:
        out = out[perm]
    return np.ascontiguousarray(out)


def prep_inputs(inputs, nsub, steps):
    """Build the 8 per-core input maps (all host-side numpy)."""
    f = lambda x: np.asarray(x, dtype=np.float32)
    latent = f(inputs["latent"])
    h_init = f(inputs["h_dec_init"])
    c_init = f(inputs["c_dec_init"])
    dW_ih0 = f(inputs["dW_ih0"])
    Wp = dW_ih0[:, :INP]
    Wp_pad = np.zeros((4 * H, INPP), np.float32)
    Wp_pad[:, :INP] = Wp
    We = dW_ih0[:, INP:]
    dOut_W = f(inputs["dOut_W"])
    dOut_W_pad = np.zeros((INPP, H), np.float32)
    dOut_W_pad[:INP] = dOut_W
    dOut_b_pad = np.zeros((1, INPP), np.float32)
    dOut_b_pad[0, :INP] = f(inputs["dOut_b"])
    cOut_W = f(inputs["cOut_W"])

    ident = np.eye(128, dtype=np.float32)
    ones = np.ones((1, B), np.float32)
    zz = np.zeros((128, 8, B), np.float32)

    in_maps = []
    for r in range(R):
        m = {
            "w_l0p": _chunk_k(_shard_gates(Wp_pad, r), 4, KP),
            "w_l0e": _chunk_k(_shard_gates(We, r), 4, 128),
            "w_l0h": _chunk_k(_shard_gates(f(inputs["dW_hh0"]), r), 8, 128),
            "w_l1a": _chunk_k(_shard_gates(f(inputs["dW_ih1"]), r), 8, 128),
            "w_l1b": _chunk_k(_shard_gates(f(inputs["dW_hh1"]), r), 8, 128),
            "w_c0l": _chunk_k(_shard_gates(f(inputs["cW_ih0"]), r), 4, 128),
            "w_c0h": _chunk_k(_shard_gates(f(inputs["cW_hh0"]), r), 8, 128),
            "w_c1a": _chunk_k(_shard_gates(f(inputs["cW_ih1"]), r), 8, 128),
            "w_c1b": _chunk_k(_shard_gates(f(inputs["cW_hh1"]), r), 8, 128),
            "b_l0": _shard_bias(f(inputs["db0"]), r),
            "b_l1": _shard_bias(f(inputs["db1"]), r),
            "b_c0": _shard_bias(f(inputs["cb0"]), r),
            "b_c1": _shard_bias(f(inputs["cb1"]), r),
            "w_out": np.ascontiguousarray(
                dOut_W_pad.T.reshape(8, 128, INPP)),
            "b_out": dOut_b_pad,
            "w_co": np.ascontiguousarray(cOut_W.T.reshape(8, 128, CO)),
            "b_co": f(inputs["cOut_b"]).reshape(1, CO),
            "latentT": np.ascontiguousarray(latent.T.reshape(4, 128, B)),
            "h_initT": np.ascontiguousarray(
                h_init[:nsub].transpose(0, 1, 3, 2).reshape(nsub, NL, 8, 128, B)),
            "c_initT": np.ascontiguousarray(
                c_init[:nsub, :, :, r * 128:(r + 1) * 128].transpose(0, 1, 3, 2)),
            "ident": ident,
            "ones": ones,
            "zz": zz,
        }
        in_maps.append(m)
    return in_maps


def build(nsub, steps, ncores=R):
    nc = bacc.Bacc("TRN2", target_bir_lowering=False, debug=False,
                   num_devices=ncores)
    seq = nsub * steps
    rg = [list(range(ncores))]

    def din(name, shape, dt=F32R):
        return nc.dram_tensor(name, list(shape), dt, kind="ExternalInput")

    d_w_l0p = din("w_l0p", [4, KP, 512])
    d_w_l0e = din("w_l0e", [4, 128, 512])
    d_w_l0h = din("w_l0h", [8, 128, 512])
    d_w_l1a = din("w_l1a", [8, 128, 512])
    d_w_l1b = din("w_l1b", [8, 128, 512])
    d_w_c0l = din("w_c0l", [4, 128, 512])
    d_w_c0h = din("w_c0h", [8, 128, 512])
    d_w_c1a = din("w_c1a", [8, 128, 512])
    d_w_c1b = din("w_c1b", [8, 128, 512])
    d_b_l0 = din("b_l0", [1, 512])
    d_b_l1 = din("b_l1", [1, 512])
    d_b_c0 = din("b_c0", [1, 512])
    d_b_c1 = din("b_c1", [1, 512])
    d_w_out = din("w_out", [8, 128, INPP])
    d_b_out = din("b_out", [1, INPP])
    d_w_co = din("w_co", [8, 128, CO])
    d_b_co = din("b_co", [1, CO])
    d_latT = din("latentT", [4, 128, B])
    d_hiT = din("h_initT", [nsub, NL, 8, 128, B])
    d_ciT = din("c_initT", [nsub, NL, 128, B])
    d_ident = din("ident", [128, 128])
    d_ones = din("ones", [1, B])
    d_zz = din("zz", [128, 8, B])
    outd = nc.dram_tensor("out", [seq, B, INPP], F32R, kind="ExternalOutput")

    with tile.TileContext(nc) as tc:
        with (
            tc.tile_pool(name="wpool", bufs=1) as wp,
            tc.tile_pool(name="state", bufs=1) as stp,
            tc.tile_pool(name="work", bufs=2) as wk,
            tc.tile_pool(name="hnew", bufs=3) as hnp,
            tc.tile_pool(name="psG", bufs=2, space="PSUM") as psG,
            tc.tile_pool(name="psT", bufs=2, space="PSUM") as psT,
            tc.tile_pool(name="psP", bufs=2, space="PSUM") as psP,
            tc.tile_pool(name="psQ", bufs=2, space="PSUM") as psQ,
            tc.tile_pool(name="dram", bufs=4, space="DRAM") as drp,
            tc.tile_pool(name="hgat", bufs=2) as hgp,
            tc.tile_pool(name="cst", bufs=2) as csp,
        ):
            # ---- load weights / constants into SBUF (resident) ----
            def load_w(dram_t, nk, kp, fshape, tag):
                t = wp.tile([kp, nk] + list(fshape), F32R, tag=tag)
                for kc in range(nk):
                    nc.sync.dma_start(t[:, kc], dram_t[kc])
                return t

            w_l0p = load_w(d_w_l0p, 4, KP, [512], "w_l0p")
            w_l0e = load_w(d_w_l0e, 4, 128, [512], "w_l0e")
            w_l0h = load_w(d_w_l0h, 8, 128, [512], "w_l0h")
            w_l1a = load_w(d_w_l1a, 8, 128, [512], "w_l1a")
            w_l1b = load_w(d_w_l1b, 8, 128, [512], "w_l1b")
            w_c0l = load_w(d_w_c0l, 4, 128, [512], "w_c0l")
            w_c0h = load_w(d_w_c0h, 8, 128, [512], "w_c0h")
            w_c1a = load_w(d_w_c1a, 8, 128, [512], "w_c1a")
            w_c1b = load_w(d_w_c1b, 8, 128, [512], "w_c1b")
            w_out = load_w(d_w_out, 8, 128, [INPP], "w_out")
            w_co = load_w(d_w_co, 8, 128, [CO], "w_co")

            def load_small(dram_t, shape, tag):
                t = wp.tile(list(shape), F32R, tag=tag)
                nc.sync.dma_start(t[:], dram_t[:])
                return t

            b_l0 = load_small(d_b_l0, [1, 512], "b_l0")
            b_l1 = load_small(d_b_l1, [1, 512], "b_l1")
            b_c0 = load_small(d_b_c0, [1, 512], "b_c0")
            b_c1 = load_small(d_b_c1, [1, 512], "b_c1")
            b_out = load_small(d_b_out, [1, INPP], "b_out")
            b_co = load_small(d_b_co, [1, CO], "b_co")
            latTs = wp.tile([128, 4, B], F32R, tag="latTs")
            for kc in range(4):
                nc.sync.dma_start(latTs[:, kc, :], d_latT[kc])
            ident = load_small(d_ident, [128, 128], "ident")
            ones = load_small(d_ones, [1, B], "ones")

            # ---- helpers ----
            def gemm_gates(psg, parts, bias=None, start=True):
                n = len(parts)
                for i, (st, wt, kc, ks) in enumerate(parts):
                    nc.tensor.matmul(
                        psg[:], st, wt[:ks, kc, :],
                        start=(start and i == 0),
                        stop=(bias is None and i == n - 1))
                if bias is not None:
                    nc.tensor.matmul(psg[:], ones[:], bias[:],
                                     start=False, stop=True)

            def allgather(own, dest):
                agin = drp.tile([128, B], F32R, tag="agin")
                agout = drp.tile([128 * R, B], F32R, tag="agout")
                nc.sync.dma_start(agin[:], own[:])
                nc.gpsimd.collective_compute(
                    "AllGather", mybir.AluOpType.bypass,
                    replica_groups=rg,
                    ins=[agin[:].opt()], outs=[agout[:].opt()])
                ag2 = agout.rearrange("(c p) b -> p c b", p=128)
                for q in range(4):
                    nc.sync.dma_start(dest[:, 2 * q:2 * q + 2, :],
                                      ag2[:, 2 * q:2 * q + 2, :])

            def gates_to_T(psg, base):
                gsb = wk.tile([B, 512], F32R, tag="gsb")
                nc.vector.tensor_add(gsb[:], psg[:], base[:])
                pst = psT.tile([128, 4, B], F32R, tag="T")
                for t4 in range(4):
                    nc.tensor.transpose(pst[:, t4, :],
                                        gsb[:, t4 * 128:(t4 + 1) * 128],
                                        ident[:B, :B])
                return pst

            def cell(pst, c_t, h_out):
                sf = wk.tile([128, 2, B], F32, tag="sf")
                tg = wk.tile([128, B], F32, tag="tg")
                so = wk.tile([128, B], F32, tag="so")
                t1 = wk.tile([128, B], F32, tag="t1")
                t2 = wk.tile([128, B], F32, tag="t2")
                tcc = wk.tile([128, B], F32, tag="tcc")
                nc.scalar.activation(sf[:], pst[:, 0:2, :], AF.Sigmoid)
                nc.scalar.activation(tg[:], pst[:, 2, :], AF.Tanh)
                nc.scalar.activation(so[:], pst[:, 3, :], AF.Sigmoid)
                nc.vector.tensor_mul(t1[:], sf[:, 0, :], tg[:])
                nc.vector.tensor_mul(t2[:], sf[:, 1, :], c_t[:])
                nc.vector.tensor_add(c_t[:], t1[:], t2[:])
                nc.scalar.activation(tcc[:], c_t[:], AF.Tanh)
                nc.vector.tensor_mul(h_out[:], so[:], tcc[:])

            def make_base(parts, bias, tag):
                psg = psG.tile([B, 512], F32, tag="G")
                if parts:
                    gemm_gates(psg, parts, bias=bias)
                else:
                    nc.tensor.matmul(psg[:], ones[:], bias[:],
                                     start=True, stop=True)
                t = stp.tile([B, 512], F32, tag=tag)
                nc.vector.tensor_copy(t[:], psg[:])
                return t

            # ---- conductor phase ----
            base_c0 = make_base(
                [(latTs[:, kc, :], w_c0l, kc, 128) for kc in range(4)],
                b_c0, "base_c0")
            base_c1 = make_base([], b_c1, "base_c1")
            base_l1 = make_base([], b_l1, "base_l1")

            hc0 = stp.tile([128, 8, B], F32R, tag="hc0")
            hc1 = stp.tile([128, 8, B], F32R, tag="hc1")
            nc.sync.dma_start(hc0[:], d_zz[:])
            nc.sync.dma_start(hc1[:], d_zz[:])
            cc0 = stp.tile([128, B], F32, tag="cc0")
            cc1 = stp.tile([128, B], F32, tag="cc1")
            nc.sync.dma_start(cc0[:], d_zz[:, 0, :].bitcast(F32))
            nc.sync.dma_start(cc1[:], d_zz[:, 1, :].bitcast(F32))

            embT = stp.tile([128, nsub, 4, B], F32R, tag="embT")
            base0 = stp.tile([B, nsub, 512], F32, tag="base0")

            for s in range(nsub):
                # conductor layer 0
                psg = psG.tile([B, 512], F32, tag="G")
                gemm_gates(psg, [(hc0[:, c, :], w_c0h, c, 128) for c in range(8)])
                pst = gates_to_T(psg, base_c0)
                h0n = hnp.tile([128, B], F32R, tag="h0n")
                cell(pst, cc0, h0n)
                allgather(h0n, hc0)
                # conductor layer 1
                psg = psG.tile([B, 512], F32, tag="G")
                gemm_gates(psg,
                           [(hc1[:, c, :], w_c1b, c, 128) for c in range(8)]
                           + [(hc0[:, c, :], w_c1a, c, 128) for c in range(8)])
                pst = gates_to_T(psg, base_c1)
                h1n = hnp.tile([128, B], F32R, tag="h1n")
                cell(pst, cc1, h1n)
                allgather(h1n, hc1)
                # emb = tanh(hc1 @ cOut_W.T + b)
                pse = psP.tile([B, 512], F32, tag="P")
                for c in range(8):
                    nc.tensor.matmul(pse[:, :CO], hc1[:, c, :], w_co[:, c, :],
                                     start=(c == 0), stop=False)
                nc.tensor.matmul(pse[:, :CO], ones[:], b_co[:], start=False,
                                 stop=True)
                emb_sb = wk.tile([B, CO], F32R, tag="emb_sb")
                nc.scalar.activation(emb_sb[:], pse[:, :CO], AF.Tanh)
                psq = psQ.tile([128, 128], F32R, tag="Q")
                for i in range(4):
                    nc.tensor.transpose(psq[:, 32 * i:32 * (i + 1)],
                                        emb_sb[:, 128 * i:128 * (i + 1)],
                                        ident[:B, :B])
                nc.vector.tensor_copy(embT[:, s], psq.rearrange("p (i b) -> p i b", b=B))
                # base0_s = emb-part of L0 gates + bias
                psg = psG.tile([B, 512], F32, tag="G")
                gemm_gates(psg, [(embT[:, s, i, :], w_l0e, i, 128) for i in range(4)],
                           bias=b_l0)
                nc.vector.tensor_copy(base0[:, s, :], psg[:])

            # ---- decoder phase ----
            prevT = stp.tile([KP, 128], F32R, tag="prevT")
            nc.sync.dma_start(prevT[:], d_zz[:KP, :4, :])

            for s in range(nsub):
                h0a = hgp.tile([128, 8, B], F32R, tag="h0a")
                h1a = hgp.tile([128, 8, B], F32R, tag="h1a")
                hi2 = d_hiT.rearrange("s l c p b -> s l p c b")
                for q in range(4):
                    nc.sync.dma_start(h0a[:, 2 * q:2 * q + 2, :],
                                      hi2[s, 0, :, 2 * q:2 * q + 2, :])
                    nc.sync.dma_start(h1a[:, 2 * q:2 * q + 2, :],
                                      hi2[s, 1, :, 2 * q:2 * q + 2, :])
                c0 = csp.tile([128, B], F32, tag="c0")
                c1 = csp.tile([128, B], F32, tag="c1")
                nc.sync.dma_start(c0[:], d_ciT[s, 0].bitcast(F32))
                nc.sync.dma_start(c1[:], d_ciT[s, 1].bitcast(F32))

                for k in range(steps):
                    tg_i = s * steps + k
                    # layer 0 gates
                    psg = psG.tile([B, 512], F32, tag="G")
                    gemm_gates(
                        psg,
                        [(prevT[:, 32 * i:32 * (i + 1)], w_l0p, i, KP)
                         for i in range(4)]
                        + [(h0a[:, c, :], w_l0h, c, 128) for c in range(8)])
                    pst = gates_to_T(psg, base0[:, s, :])
                    h0n = hnp.tile([128, B], F32R, tag="h0n")
                    cell(pst, c0, h0n)
                    allgather(h0n, h0a)
                    # layer 1 gates (h1-part first: overlaps the h0 allgather)
                    psg = psG.tile([B, 512], F32, tag="G")
                    gemm_gates(
                        psg,
                        [(h1a[:, c, :], w_l1b, c, 128) for c in range(8)]
                        + [(h0a[:, c, :], w_l1a, c, 128) for c in range(8)])
                    pst = gates_to_T(psg, base_l1)
                    h1n = hnp.tile([128, B], F32R, tag="h1n")
                    cell(pst, c1, h1n)
                    allgather(h1n, h1a)
                    # prev = tanh(h1 @ dOut_W.T + b), computed fully per core
                    psp = psP.tile([B, 512], F32, tag="P")
                    for c in range(8):
                        nc.tensor.matmul(psp[:, :INPP], h1a[:, c, :],
                                         w_out[:, c, :],
                                         start=(c == 0), stop=False)
                    nc.tensor.matmul(psp[:, :INPP], ones[:], b_out[:],
                                     start=False, stop=True)
                    prev_sb = wk.tile([B, INPP], F32R, tag="prev_sb")
                    nc.scalar.activation(prev_sb[:], psp[:, :INPP], AF.Tanh)
                    nc.sync.dma_start(outd[tg_i], prev_sb[:])
                    psq = psQ.tile([128, 128], F32R, tag="Q")
                    for i in range(4):
                        nc.tensor.transpose(psq[:KP, 32 * i:32 * (i + 1)],
                                            prev_sb[:, KP * i:KP * (i + 1)],
                                            ident[:B, :B])
                    nc.vector.tensor_copy(prevT[:], psq[:KP, :])

    nc.compile()
    return nc


_CACHE = {}


def _get_nc(nsub, steps):
    key = (nsub, steps)
    if key not in _CACHE:
        _CACHE[key] = build(nsub, steps)
    return _CACHE[key]


def run(inputs, nsub=NSUB, steps=STEPS, **kw):
    nc = _get_nc(nsub, steps)
    in_maps = prep_inputs(inputs, nsub, steps)
    res = bass_utils.run_bass_kernel_spmd(nc, in_maps,
                                          core_ids=list(range(R)), **kw)
    out = res.results[0]["out"]  # (seq, B, INPP)
    out_full = np.ascontiguousarray(
        out[:, :, :INP].transpose(1, 0, 2)).astype(np.float32)
    return out_full, res


def kernel(**inputs):
    out, _ = run(inputs)
    return out


# revision 15
# speedup vs baseline: 3.4146x; 3.4146x over previous
"""Trainium2 Bass kernel for hierarchical LSTM decoder (MusicVAE-style).

Strategy: tensor-parallel over the 4H gate dimension across 8 cores
(512 gate rows = 128 hidden units per core per layer). All recurrent
weights stay resident in SBUF (they do not fit on fewer cores in fp32).
Per recurrent step: each core computes gates for its hidden chunk with
float32r matmuls (activations stationary), applies the LSTM cell in
transposed (hidden-major) layout, then all-gathers the 128x32 h-chunk
so every core has the full hidden state for the next contraction.
The conductor LSTM (depends only on `latent`) is computed up front the
same way; its `emb` outputs and the per-subsequence emb/bias gate
contributions are precomputed into base tiles. The feedback projection
`prev` is computed redundantly on every core from the gathered h1.

Two all-gather mechanisms:
  mech='cc': ncfw AllGather collectives through DRAM bounce buffers.
  mech='rd': direct SBUF->SBUF remote-DMA broadcasts. Each core sends
    its chunk to relative destination k (tpb XOR k) writing slot k, so
    receiver r's slot k holds core (r^k)'s chunk; the per-core weight
    K-chunk order is XOR-permuted on the host to match. Arrival is
    signalled on pinned semaphores; the waits are attached to consumer
    matmuls AFTER Tile scheduling (the single-core scheduling sim cannot
    satisfy cross-core semaphores). Gather buffers are double-buffered
    by round parity; senders cannot run more than one round ahead
    (their next round needs our chunk), which bounds writer/reader skew.
"""
import sys
import numpy as np

sys.path.insert(0, "/opt/trn_rl_repo")

import concourse.bass as bass  # noqa: E402
import concourse.bacc as bacc  # noqa: E402
import concourse.mybir as mybir  # noqa: E402
from concourse import tile  # noqa: E402
from concourse import bass_utils  # noqa: E402

F32R = mybir.dt.float32r
F32 = mybir.dt.float32
AF = mybir.ActivationFunctionType

B, LAT, CH, CO, INP, H, SEQ, NSUB, NL = 32, 512, 1024, 512, 389, 1024, 128, 8, 2
STEPS = SEQ // NSUB
INPP = 392  # INP padded to 4*98
R = 8  # cores / TP degree
KP = 98  # prev-part K chunk size
MECH = "cc"


def _shard_gates(W, r):
    """W (4096, K) -> dev (K, 512) for core r: device col t*128+n <- gate
    row t*1024 + r*128 + n (transpose block t = gate t's hidden/batch tile)."""
    K = W.shape[1]
    out = np.empty((K, 512), np.float32)
    for t in range(4):
        blk = W[t * 1024 + r * 128: t * 1024 + (r + 1) * 128, :]
        out[:, t * 128:(t + 1) * 128] = blk.T
    return out


def _shard_bias(b, r):
    out = np.empty((1, 512), np.float32)
    for t in range(4):
        out[0, t * 128:(t + 1) * 128] = b[t * 1024 + r * 128: t * 1024 + r * 128 + 128]
    return out


def _chunk_k(dev, nk, kp, perm=None):
    K = dev.shape[0]
    assert K == nk * kp
    out = dev.reshape((nk, kp) + dev.shape[1:])
    if perm is not None:
        out = out[perm]
    return np.ascontiguousarray(out)


def prep_inputs(inputs, nsub, steps, mech=MECH):
    f = lambda x: np.asarray(x, dtype=np.float32)
    latent = f(inputs["latent"])
    h_init = f(inputs["h_dec_init"])
    c_init = f(inputs["c_dec_init"])
    dW_ih0 = f(inputs["dW_ih0"])
    Wp_pad = np.zeros((4 * H, INPP), np.float32)
    Wp_pad[:, :INP] = dW_ih0[:, :INP]
    We = dW_ih0[:, INP:]
    dOut_W_pad = np.zeros((INPP, H), np.float32)
    dOut_W_pad[:INP] = f(inputs["dOut_W"])
    dOut_b_pad = np.zeros((1, INPP), np.float32)
    dOut_b_pad[0, :INP] = f(inputs["dOut_b"])

    ident = np.eye(128, dtype=np.float32)
    ones = np.ones((1, B), np.float32)
    zz = np.zeros((128, 8, B), np.float32)
    hiT_full = h_init[:nsub].transpose(0, 1, 3, 2).reshape(nsub, NL, 8, 128, B)

    in_maps = []
    for r in range(R):
        perm = ([r ^ k ^ (2 if k >= 4 else 0) for k in range(8)]
                if mech == "rd" else None)
        m = {
            "w_l0p": _chunk_k(_shard_gates(Wp_pad, r), 4, KP),
            "w_l0e": _chunk_k(_shard_gates(We, r), 4, 128),
            "w_l0h": _chunk_k(_shard_gates(f(inputs["dW_hh0"]), r), 8, 128, perm),
            "w_l1a": _chunk_k(_shard_gates(f(inputs["dW_ih1"]), r), 8, 128, perm),
            "w_l1b": _chunk_k(_shard_gates(f(inputs["dW_hh1"]), r), 8, 128, perm),
            "w_c0l": _chunk_k(_shard_gates(f(inputs["cW_ih0"]), r), 4, 128),
            "w_c0h": _chunk_k(_shard_gates(f(inputs["cW_hh0"]), r), 8, 128, perm),
            "w_c1a": _chunk_k(_shard_gates(f(inputs["cW_ih1"]), r), 8, 128, perm),
            "w_c1b": _chunk_k(_shard_gates(f(inputs["cW_hh1"]), r), 8, 128, perm),
            "b_l0": _shard_bias(f(inputs["db0"]), r),
            "b_l1": _shard_bias(f(inputs["db1"]), r),
            "b_c0": _shard_bias(f(inputs["cb0"]), r),
            "b_c1": _shard_bias(f(inputs["cb1"]), r),
            "w_out": _chunk_k(np.ascontiguousarray(dOut_W_pad.T), 8, 128, perm),
            "b_out": dOut_b_pad,
            "w_co": _chunk_k(np.ascontiguousarray(f(inputs["cOut_W"]).T), 8, 128, perm),
            "b_co": f(inputs["cOut_b"]).reshape(1, CO),
            "latentT": np.ascontiguousarray(latent.T.reshape(4, 128, B)),
            "h_initT": np.ascontiguousarray(
                hiT_full[:, :, perm] if perm is not None else hiT_full),
            "c_initT": np.ascontiguousarray(
                c_init[:nsub, :, :, r * 128:(r + 1) * 128].transpose(0, 1, 3, 2)),
            "ident": ident,
            "ones": ones,
            "zz": zz,
        }
        in_maps.append(m)
    return in_maps


def build(nsub, steps, ncores=R, mech=MECH):
    nc = bacc.Bacc("TRN2", target_bir_lowering=False, debug=False,
                   num_devices=ncores)
    seq = nsub * steps
    rg = [list(range(ncores))]

    def din(name, shape):
        return nc.dram_tensor(name, list(shape), F32R, kind="ExternalInput")

    d_w_l0p = din("w_l0p", [4, KP, 512])
    d_w_l0e = din("w_l0e", [4, 128, 512])
    d_w_l0h = din("w_l0h", [8, 128, 512])
    d_w_l1a = din("w_l1a", [8, 128, 512])
    d_w_l1b = din("w_l1b", [8, 128, 512])
    d_w_c0l = din("w_c0l", [4, 128, 512])
    d_w_c0h = din("w_c0h", [8, 128, 512])
    d_w_c1a = din("w_c1a", [8, 128, 512])
    d_w_c1b = din("w_c1b", [8, 128, 512])
    d_b_l0 = din("b_l0", [1, 512])
    d_b_l1 = din("b_l1", [1, 512])
    d_b_c0 = din("b_c0", [1, 512])
    d_b_c1 = din("b_c1", [1, 512])
    d_w_out = din("w_out", [8, 128, INPP])
    d_b_out = din("b_out", [1, INPP])
    d_w_co = din("w_co", [8, 128, CO])
    d_b_co = din("b_co", [1, CO])
    d_latT = din("latentT", [4, 128, B])
    d_hiT = din("h_initT", [nsub, NL, 8, 128, B])
    d_ciT = din("c_initT", [nsub, NL, 128, B])
    d_ident = din("ident", [128, 128])
    d_ones = din("ones", [1, B])
    d_zz = din("zz", [128, 8, B])
    outd = nc.dram_tensor("out", [seq, B, INPP], F32R, kind="ExternalOutput")

    rd = mech == "rd"
    if rd:
        rsems = {nm: nc.alloc_semaphore(f"agr_{nm}")
                 for nm in ("h0", "h1", "c0", "c1")}
        lsems = {nm: nc.alloc_semaphore(f"agl_{nm}")
                 for nm in ("h0", "h1", "c0", "c1")}
    late_waits = []  # (BassInstruction, sem, val) attached post-scheduling
    PAR = 2 if rd else 1

    with tile.TileContext(nc) as tc:
        with (
            tc.tile_pool(name="wpool", bufs=1) as wp,
            tc.tile_pool(name="state", bufs=1) as stp,
            tc.tile_pool(name="work", bufs=2) as wk,
            tc.tile_pool(name="hnew", bufs=3) as hnp,
            tc.tile_pool(name="psG", bufs=2, space="PSUM") as psG,
            tc.tile_pool(name="psT", bufs=2, space="PSUM") as psT,
            tc.tile_pool(name="psP", bufs=2, space="PSUM") as psP,
            tc.tile_pool(name="psQ", bufs=2, space="PSUM") as psQ,
            tc.tile_pool(name="dram", bufs=4, space="DRAM") as drp,
        ):
            def load_w(pool, dram_t, nk, kp, fshape, tag):
                t = pool.tile([kp, nk] + list(fshape), F32R, tag=tag)
                for kc in range(nk):
                    nc.sync.dma_start(t[:, kc], dram_t[kc])
                return t

            w_l0p = load_w(wp, d_w_l0p, 4, KP, [512], "w_l0p")
            w_l0h = load_w(wp, d_w_l0h, 8, 128, [512], "w_l0h")
            w_l1a = load_w(wp, d_w_l1a, 8, 128, [512], "w_l1a")
            w_l1b = load_w(wp, d_w_l1b, 8, 128, [512], "w_l1b")
            w_out = load_w(wp, d_w_out, 8, 128, [INPP], "w_out")

            def load_small(pool, dram_t, shape, tag):
                t = pool.tile(list(shape), F32R, tag=tag)
                nc.sync.dma_start(t[:], dram_t[:])
                return t

            b_out = load_small(wp, d_b_out, [1, INPP], "b_out")
            ident = load_small(wp, d_ident, [128, 128], "ident")
            ones = load_small(wp, d_ones, [1, B], "ones")

            def gemm_gates(psg, parts, bias=None, waits=()):
                n = len(parts)
                for i, (st, wt, kc, ks) in enumerate(parts):
                    mm = nc.tensor.matmul(
                        psg[:], st, wt[:ks, kc, :],
                        start=(i == 0),
                        stop=(bias is None and i == n - 1))
                    for sem, val, pred in waits:
                        if pred(i):
                            late_waits.append((mm, sem, val))
                if bias is not None:
                    nc.tensor.matmul(psg[:], ones[:], bias[:],
                                     start=False, stop=True)

            def allgather(own, dest, stream):
                """own (128,B) f32r -> dest slot k on core (self^k)."""
                if rd:
                    for k in range(ncores):
                        rdl = [None] * 8
                        rdl[k] = (0, k)
                        nc.gpsimd.remote_dma_broadcast(
                            out_ap=dest[:, k, :], in_ap=own[:],
                            remote_sem=rsems[stream], local_sem=lsems[stream],
                            rdests=rdl)
                    nc.gpsimd.trigger_dma(count=None)
                else:
                    agin = drp.tile([128, B], F32R, tag="agin")
                    agout = drp.tile([128 * ncores, B], F32R, tag="agout")
                    nc.sync.dma_start(agin[:], own[:])
                    nc.gpsimd.collective_compute(
                        "AllGather", mybir.AluOpType.bypass,
                        replica_groups=rg,
                        ins=[agin[:].opt()], outs=[agout[:].opt()])
                    ag2 = agout.rearrange("(c p) b -> p c b", p=128)
                    for q in range(4):
                        nc.sync.dma_start(dest[:, 2 * q:2 * q + 2, :],
                                          ag2[:, 2 * q:2 * q + 2, :])

            def gates_to_T(psg, base):
                gsb = wk.tile([B, 512], F32R, tag="gsb")
                pst = psT.tile([128, 4, B], F32R, tag="T")
                for t4 in range(4):
                    sl = slice(t4 * 128, (t4 + 1) * 128)
                    nc.vector.tensor_add(gsb[:, sl], psg[:, sl], base[:, sl])
                    nc.tensor.transpose(pst[:, t4, :], gsb[:, sl],
                                        ident[:B, :B])
                return pst

            def cell(pst, c_t, h_out, stream=None, rnd=0):
                sf = wk.tile([128, 2, B], F32, tag="sf")
                tg = wk.tile([128, B], F32, tag="tg")
                so = wk.tile([128, B], F32, tag="so")
                t1 = wk.tile([128, B], F32, tag="t1")
                t2 = wk.tile([128, B], F32, tag="t2")
                tcc = wk.tile([128, B], F32, tag="tcc")
                nc.scalar.activation(sf[:], pst[:, 0:2, :], AF.Sigmoid)
                nc.scalar.activation(tg[:], pst[:, 2, :], AF.Tanh)
                nc.scalar.activation(so[:], pst[:, 3, :], AF.Sigmoid)
                nc.vector.tensor_mul(t1[:], sf[:, 0, :], tg[:])
                nc.vector.tensor_mul(t2[:], sf[:, 1, :], c_t[:])
                nc.vector.tensor_add(c_t[:], t1[:], t2[:])
                nc.scalar.activation(tcc[:], c_t[:], AF.Tanh)
                hm = nc.vector.tensor_mul(h_out[:], so[:], tcc[:])
                # h_out slot reuse guard (pool bufs=3): sends of round rnd-3
                # must have left this core before we overwrite its buffer.
                if rd and stream is not None and rnd >= 3:
                    late_waits.append((hm, lsems[stream], 128 * (rnd - 2)))

            def make_base(parts, bias, tag, pool=None):
                psg = psG.tile([B, 512], F32, tag="G")
                if parts:
                    gemm_gates(psg, parts, bias=bias)
                else:
                    nc.tensor.matmul(psg[:], ones[:], bias[:],
                                     start=True, stop=True)
                t = (pool or stp).tile([B, 512], F32, tag=tag)
                nc.vector.tensor_copy(t[:], psg[:])
                return t

            def rw(stream, rnd, pred):
                """wait for arrival of gather round rnd (16 incs/round)"""
                if not rd or rnd < 0:
                    return ()
                return ((rsems[stream], 16 * (rnd + 1), pred),)

            # ---- conductor phase (weights live in a released pool) ----
            embT = stp.tile([128, nsub, 4, B], F32R, tag="embT")
            base0 = stp.tile([B, nsub, 512], F32, tag="base0")
            ALL = lambda i: True
            cw = tc.tile_pool(name="cwpool", bufs=1)
            cwp = cw.__enter__()
            w_l0e = load_w(cwp, d_w_l0e, 4, 128, [512], "w_l0e")
            w_c0l = load_w(cwp, d_w_c0l, 4, 128, [512], "w_c0l")
            w_c0h = load_w(cwp, d_w_c0h, 8, 128, [512], "w_c0h")
            w_c1a = load_w(cwp, d_w_c1a, 8, 128, [512], "w_c1a")
            w_c1b = load_w(cwp, d_w_c1b, 8, 128, [512], "w_c1b")
            w_co = load_w(cwp, d_w_co, 8, 128, [CO], "w_co")
            b_l0 = load_small(cwp, d_b_l0, [1, 512], "b_l0")
            b_l1 = load_small(cwp, d_b_l1, [1, 512], "b_l1")
            b_c0 = load_small(cwp, d_b_c0, [1, 512], "b_c0")
            b_c1 = load_small(cwp, d_b_c1, [1, 512], "b_c1")
            b_co = load_small(cwp, d_b_co, [1, CO], "b_co")
            latTs = cwp.tile([128, 4, B], F32R, tag="latTs")
            for kc in range(4):
                nc.sync.dma_start(latTs[:, kc, :], d_latT[kc])

            base_c0 = make_base(
                [(latTs[:, kc, :], w_c0l, kc, 128) for kc in range(4)],
                b_c0, "base_c0", pool=cwp)
            base_c1 = make_base([], b_c1, "base_c1", pool=cwp)
            base_l1 = make_base([], b_l1, "base_l1")

            hc0 = cwp.tile([128, PAR, 8, B], F32R, tag="hc0")
            hc1 = cwp.tile([128, PAR, 8, B], F32R, tag="hc1")
            ipc = PAR - 1  # init parity slot ((-1) % 2)
            nc.sync.dma_start(hc0[:, ipc], d_zz[:])
            nc.sync.dma_start(hc1[:, ipc], d_zz[:])
            cc0 = cwp.tile([128, B], F32, tag="cc0")
            cc1 = cwp.tile([128, B], F32, tag="cc1")
            nc.sync.dma_start(cc0[:], d_zz[:, 0, :].bitcast(F32))
            nc.sync.dma_start(cc1[:], d_zz[:, 1, :].bitcast(F32))

            for s in range(nsub):
                rp = (s - 1) % PAR if s > 0 else ipc
                wpar = s % PAR
                # conductor layer 0: reads hc0 round s-1
                psg = psG.tile([B, 512], F32, tag="G")
                gemm_gates(psg, [(hc0[:, rp, c, :], w_c0h, c, 128)
                                 for c in range(8)],
                           waits=rw("c0", s - 1, ALL) if s > 0 else ())
                pst = gates_to_T(psg, base_c0)
                h0n = hnp.tile([128, B], F32R, tag="hc0n")
                cell(pst, cc0, h0n, "c0", s)
                allgather(h0n, hc0[:, wpar], "c0")
                # conductor layer 1: h1-part (round s-1), then h0-part (round s)
                psg = psG.tile([B, 512], F32, tag="G")
                parts = ([(hc1[:, rp, c, :], w_c1b, c, 128) for c in range(8)]
                         + [(hc0[:, wpar, c, :], w_c1a, c, 128) for c in range(8)])
                wts = (rw("c1", s - 1, lambda i: i < 8) if s > 0 else ()) + \
                    rw("c0", s, lambda i: i >= 8)
                gemm_gates(psg, parts, waits=wts)
                pst = gates_to_T(psg, base_c1)
                h1n = hnp.tile([128, B], F32R, tag="hc1n")
                cell(pst, cc1, h1n, "c1", s)
                allgather(h1n, hc1[:, wpar], "c1")
                # emb from gathered hc1 round s
                pse = psP.tile([B, 512], F32, tag="P")
                for c in range(8):
                    mm = nc.tensor.matmul(pse[:, :CO], hc1[:, wpar, c, :],
                                          w_co[:, c, :],
                                          start=(c == 0), stop=False)
                    for sem, val, _ in rw("c1", s, ALL):
                        late_waits.append((mm, sem, val))
                nc.tensor.matmul(pse[:, :CO], ones[:], b_co[:], start=False,
                                 stop=True)
                emb_sb = wk.tile([B, CO], F32R, tag="emb_sb")
                nc.scalar.activation(emb_sb[:], pse[:, :CO], AF.Tanh)
                psq = psQ.tile([128, 128], F32R, tag="Q")
                for i in range(4):
                    nc.tensor.transpose(psq[:, 32 * i:32 * (i + 1)],
                                        emb_sb[:, 128 * i:128 * (i + 1)],
                                        ident[:B, :B])
                nc.vector.tensor_copy(embT[:, s],
                                      psq.rearrange("p (i b) -> p i b", b=B))
                psg = psG.tile([B, 512], F32, tag="G")
                gemm_gates(psg, [(embT[:, s, i, :], w_l0e, i, 128)
                                 for i in range(4)], bias=b_l0)
                nc.vector.tensor_copy(base0[:, s, :], psg[:])

            cw.__exit__(None, None, None)

            # ---- decoder phase ----
            hg_ctx = tc.tile_pool(name="hgat", bufs=2)
            cs_ctx = tc.tile_pool(name="cst", bufs=2)
            hgp = hg_ctx.__enter__()
            csp = cs_ctx.__enter__()
            prevT = stp.tile([KP, 128], F32R, tag="prevT")
            nc.sync.dma_start(prevT[:], d_zz[:KP, :4, :])

            for s in range(nsub):
                h0a = hgp.tile([128, PAR, 8, B], F32R, tag="h0a")
                h1a = hgp.tile([128, PAR, 8, B], F32R, tag="h1a")
                ip = (s * steps - 1) % PAR  # init parity slot
                hi2 = d_hiT.rearrange("s l c p b -> s l p c b")
                for q in range(4):
                    nc.sync.dma_start(h0a[:, ip, 2 * q:2 * q + 2, :],
                                      hi2[s, 0, :, 2 * q:2 * q + 2, :])
                    nc.sync.dma_start(h1a[:, ip, 2 * q:2 * q + 2, :],
                                      hi2[s, 1, :, 2 * q:2 * q + 2, :])
                c0 = csp.tile([128, B], F32, tag="c0")
                c1 = csp.tile([128, B], F32, tag="c1")
                nc.sync.dma_start(c0[:], d_ciT[s, 0].bitcast(F32))
                nc.sync.dma_start(c1[:], d_ciT[s, 1].bitcast(F32))

                # L0 h0-part for k=0 (reads the init slot), pipelined ahead
                psg0 = psG.tile([B, 512], F32, tag="G")
                for c in range(8):
                    nc.tensor.matmul(psg0[:], h0a[:, ip, c, :],
                                     w_l0h[:, c, :], start=(c == 0),
                                     stop=False, skip_group_check=True)

                for k in range(steps):
                    n = s * steps + k
                    rp = (n - 1) % PAR if k > 0 else ip
                    wpar = n % PAR
                    # layer 0: close the group with the prev-part
                    psg = psg0
                    for i in range(4):
                        nc.tensor.matmul(
                            psg[:], prevT[:, 32 * i:32 * (i + 1)],
                            w_l0p[:KP, i, :], start=False, stop=(i == 3),
                            skip_group_check=True)
                    pst = gates_to_T(psg, base0[:, s, :])
                    h0n = hnp.tile([128, B], F32R, tag="h0n")
                    cell(pst, c0, h0n, "h0", n)
                    allgather(h0n, h0a[:, wpar], "h0")
                    # layer 1: h1-part first (overlaps h0 gather), then h0-part
                    psg = psG.tile([B, 512], F32, tag="G")
                    parts = ([(h1a[:, rp, c, :], w_l1b, c, 128)
                              for c in range(8)]
                             + [(h0a[:, wpar, c, :], w_l1a, c, 128)
                                for c in range(8)])
                    wts = (rw("h1", n - 1, lambda i: i < 8) if k > 0 else ()) + \
                        rw("h0", n, lambda i: i >= 8)
                    gemm_gates(psg, parts, waits=wts)
                    pst = gates_to_T(psg, base_l1)
                    h1n = hnp.tile([128, B], F32R, tag="h1n")
                    cell(pst, c1, h1n, "h1", n)
                    allgather(h1n, h1a[:, wpar], "h1")
                    # next step's L0 h0-part: fills the h1-gather latency
                    if k + 1 < steps:
                        psg0 = psG.tile([B, 512], F32, tag="G")
                        for c in range(8):
                            mm = nc.tensor.matmul(
                                psg0[:], h0a[:, wpar, c, :], w_l0h[:, c, :],
                                start=(c == 0), stop=False,
                                skip_group_check=True)
                            for sem, val, _ in rw("h0", n, ALL):
                                late_waits.append((mm, sem, val))
                    # prev = tanh(h1 @ dOut_W.T + b) from gathered round n
                    psp = psP.tile([B, 512], F32, tag="P")
                    for c in range(8):
                        mm = nc.tensor.matmul(psp[:, :INPP], h1a[:, wpar, c, :],
                                              w_out[:, c, :],
                                              start=(c == 0), stop=False)
                        for sem, val, _ in rw("h1", n, ALL):
                            late_waits.append((mm, sem, val))
                    nc.tensor.matmul(psp[:, :INPP], ones[:], b_out[:],
                                     start=False, stop=True)
                    prev_sb = wk.tile([B, INPP], F32R, tag="prev_sb")
                    nc.scalar.activation(prev_sb[:], psp[:, :INPP], AF.Tanh)
                    nc.sync.dma_start(outd[n], prev_sb[:])
                    psq = psQ.tile([128, 128], F32R, tag="Q")
                    for i in range(4):
                        nc.tensor.transpose(psq[:KP, 32 * i:32 * (i + 1)],
                                            prev_sb[:, KP * i:KP * (i + 1)],
                                            ident[:B, :B])
                    nc.vector.tensor_copy(prevT[:], psq[:KP, :])

            cs_ctx.__exit__(None, None, None)
            hg_ctx.__exit__(None, None, None)

    for inst, sem, val in late_waits:
        inst.wait_op(sem, val, "sem-ge", check=False)

    nc.compile()
    return nc


_CACHE = {}


def _get_nc(nsub, steps, mech=MECH):
    key = (nsub, steps, mech)
    if key not in _CACHE:
        _CACHE[key] = build(nsub, steps, mech=mech)
    return _CACHE[key]


def run(inputs, nsub=NSUB, steps=STEPS, mech=MECH, **kw):
    nc = _get_nc(nsub, steps, mech)
    in_maps = prep_inputs(inputs, nsub, steps, mech)
    res = bass_utils.run_bass_kernel_spmd(nc, in_maps,
                                          core_ids=list(range(R)), **kw)
    out = res.results[0]["out"]  # (seq, B, INPP)
    out_full = np.ascontiguousarray(
        out[:, :, :INP].transpose(1, 0, 2)).astype(np.float32)
    return out_full, res


def kernel(**inputs):
    out, _ = run(inputs)
    return out
